# revision 1
# baseline (speedup 1.0000x reference)
"""Trainium2 Bass kernel for nn_BatchMuSc (retrieval_knn).

Computes, for Z [96, 256, 128] and cls_tokens [96, 768]:
  - MSM patch anomaly scores: for each image i, for each of its 256 patches,
    the mean of the 28 smallest per-reference-image minimal euclidean
    distances to all other images' patches.
  - img_scores = max over patches; min-max normalize.
  - RsCIN/MMO refinement with W = cls @ cls.T, top-k row masks (k=1,2,3).
  Output: [96] float32.

Strategy (8 NeuronCores, data-parallel over query images):
  - Every core receives the full Z, rolled by -12*core images, so its 12
    query images are always local images 0..11 (static addressing; SPMD).
  - Per core: ZT [128(C), 24576] resident in SBUF (fp32r), distances via
    PSUM-accumulated fp32r matmuls: B = 2*q.z - |z|^2 (rank-1 adds -|z|^2),
    grouped max-reduce over each reference image's 256 patches
    (max B = -min d2), then top-28-smallest via max8/match_replace.
  - img_scores are AllGathered across cores; every core redundantly runs the
    tiny MMO refinement; core 0's output is returned.
"""
import os
import sys
import types

import numpy as np

for _p in ("/opt/trn_rl_repo",):
    if _p not in sys.path and os.path.isdir(_p):
        sys.path.insert(0, _p)

# The axon NTFF profile hook module is absent in this environment; stub it so
# run_bass_kernel_spmd can import it (only needed for trace=True).
try:  # pragma: no cover
    import antenv.axon_hooks  # noqa: F401
except Exception:  # pragma: no cover
    _m = types.ModuleType("antenv.axon_hooks")
    _m.get_axon_ntff_profile_hook = lambda: None
    sys.modules["antenv.axon_hooks"] = _m

import concourse.bacc as bacc
import concourse.bass_isa as bass_isa
import concourse.mybir as mybir
from concourse import bass_utils
from concourse.masks import make_identity
from concourse.tile import TileContext

F32 = mybir.dt.float32
F32R = mybir.dt.float32r
FP16 = mybir.dt.float16
AX = mybir.AxisListType.X
OP = mybir.AluOpType
ACTF = mybir.ActivationFunctionType

N, L, C, DC = 96, 256, 128, 768
NCORES = 8
IPC = N // NCORES          # 12 query images per core
NL = N * L                 # 24576 total patches
NT = NL // 128             # 192 transpose tiles
NS = NL // 512             # 48 stripes of 512 patches (2 images each)
NQ = NS // 4               # 12 quads of 4 stripes (8 images each)
KTOP = 28                  # int((N-1)*0.3) smallest distances averaged
EPS = 1e-12
NEG = -3.4e38

# Per-quad reduction path: D = DVE reduce from PSUM;
# A = ACT copy to SBUF + DVE reduce; G = ACT copy + GPSIMD pairwise fold +
# DVE short reduce.  Tunable (len NQ).
# Per-quad paths:
#  D = DVE reduce direct from PSUM (needs rank-1)
#  A = ACT copy to SBUF + DVE flat reduce (needs rank-1)
#  R = rank-1 on PE + ACT copy + DVE pairwise-max tree
#  C = no rank-1: ACT copy + DVE tensor-add of -|z|^2 + pairwise-max tree
QUAD_PATHS = os.environ.get("BMS_QUAD_PATHS", "RRRDRRRDRRRD")
assert len(QUAD_PATHS) == NQ and set(QUAD_PATHS) <= set("DARC")
MM_DT = os.environ.get("BMS_MM_DT", "fp16")       # matmul operand dtype
RED_DT = os.environ.get("BMS_RED_DT", "fp16")     # copy/reduce dtype


def build(
    quad_paths: str = QUAD_PATHS,
    repeat_main: int = 1,
    n_cores: int = NCORES,
    stop: str = "full",
    dummy: bool = False,
    ablate: str = "",
    mm_dt: str = None,
    red_dt: str = None,
):
    mm_dt = MM_DT if mm_dt is None else mm_dt
    red_dt = RED_DT if red_dt is None else red_dt
    MDT = {"f32r": F32R, "fp16": FP16}[mm_dt]
    RDT = {"f32": F32, "fp16": FP16}[red_dt]
    nc = bacc.Bacc(
        "TRN2",
        target_bir_lowering=False,
        debug=False,
        enable_asserts=False,
        num_devices=n_cores,
    )
    Z = None if dummy else nc.dram_tensor("Z", [N, L, C], F32, kind="ExternalInput")
    cls = nc.dram_tensor("cls_tokens", [N, DC], F32, kind="ExternalInput")
    out = nc.dram_tensor("out", [N], F32, kind="ExternalOutput")
    cc_in = nc.dram_tensor("cc_in", [IPC], F32, kind="Internal")
    cc_out = nc.dram_tensor("cc_out", [N], F32, kind="Internal", addr_space="Shared")

    stages = ["p0", "p0b", "p1", "p2", "full"]
    sidx = stages.index(stop)
    with TileContext(nc) as tc:
        with tc.tile_pool(name="persist", bufs=1) as pers:
            ident = pers.tile([128, 128], F32)
            make_identity(nc, ident)
            ones_f = pers.tile([128, 128], F32)
            nc.vector.memset(ones_f, 1.0)
            ones_r = pers.tile([128, 128], MDT)
            nc.vector.tensor_copy(ones_r, ones_f)
            epsb = pers.tile([128, 1], F32)
            nc.vector.memset(epsb, EPS)
            oinv_f = pers.tile([128, 128], F32)
            nc.vector.memset(oinv_f, 1.0 / 128.0)
            ones_inv = pers.tile([128, 128], MDT)
            nc.vector.tensor_copy(ones_inv, oinv_f)

            ZT = pers.tile([128, NL], MDT)           # channels x patches
            sq_q = pers.tile([128, 2 * IPC], F32)    # |z|^2 of local queries
            # -|z_p|^2 packed for rank-1 matmul rhs reads. Matmul operands
            # must start at partition 0/32/64, so stripes live on exactly
            # those three rows: row 32*(s//16), columns 512*(s%16).
            nsq = pers.tile([65, 16 * 512], MDT)
            # negated |z_p|^2 replicated across partitions; used both as the
            # rank-128 matmul rhs (adds -|z|^2 into PSUM at full matmul speed:
            # lhsT is the constant 1/128 matrix, and summing 128 identical
            # fp16 values of v/128 in fp32 PSUM reconstructs v exactly) and
            # for C-path DVE adds.
            nsq_rep = pers.tile([128, NL], MDT)
            score_all = pers.tile([128, 2 * IPC], F32)
            simg = pers.tile([1, N], F32)

            # ---- Phase 0: load Z, build ZT (transposed, fp32r), query norms
            if dummy:
                if mm_dt == "f32r":
                    nc.vector.memset(ZT.bitcast(F32), 0.5)
                    nc.vector.memset(nsq.bitcast(F32), -32.0)
                else:
                    nc.vector.memset(ZT, 0.5)
                    nc.vector.memset(nsq, -32.0)
                nc.vector.memset(sq_q, float(C) * 0.25)
                nc.vector.memset(nsq_rep, -32.0)
            else:
              Zf = Z.ap().rearrange("n l c -> (n l) c")
              with (
                tc.tile_pool(name="stage", bufs=4) as stage,
                tc.tile_pool(name="tpsum", bufs=8, space="PSUM") as tps,
                tc.tile_pool(name="sqscr", bufs=2) as sqscr,
              ):
                for t in range(NT):
                    st = stage.tile([128, C], F32, tag="st")
                    nc.sync.dma_start(st, Zf[128 * t : 128 * (t + 1), :])
                    pt = tps.tile([128, 128], F32, tag="pt")
                    nc.tensor.transpose(pt, st, ident)
                    dst = ZT[:, 128 * t : 128 * (t + 1)]
                    if t % 2 == 0:
                        nc.scalar.copy(dst, pt)
                    else:
                        nc.vector.tensor_copy(dst, pt)
                    if t < 2 * IPC:
                        dm = sqscr.tile([128, C], F32, tag="dm")
                        nc.scalar.activation(
                            dm, st, ACTF.Square, accum_out=sq_q[:, t : t + 1]
                        )

            # ---- Phase 0b: negated patch norms -|z_p|^2 in rank-1 layout
            if sidx >= 1:
              with (
                tc.tile_pool(name="z2p", bufs=3) as z2p,
                tc.tile_pool(name="sqpsum", bufs=4, space="PSUM") as sqp,
              ):
                for s in range(NS):
                    z2 = z2p.tile([128, 512], MDT, tag="z2")
                    nc.vector.tensor_mul(
                        z2, ZT[:, 512 * s : 512 * (s + 1)], ZT[:, 512 * s : 512 * (s + 1)]
                    )
                    psq = sqp.tile([128, 512], F32, tag="psq")
                    nc.tensor.matmul(psq, lhsT=ones_r, rhs=z2, start=True, stop=True)
                    # every psq row holds the same column sums; copy from the
                    # partition row matching nsq's layout
                    row = 32 * (s // 16)
                    off = 512 * (s % 16)
                    nc.scalar.mul(
                        nsq[row : row + 1, off : off + 512],
                        psq[row : row + 1, :],
                        -1.0,
                    )
                    nc.scalar.mul(
                        nsq_rep[:, 512 * s : 512 * (s + 1)], psq, -1.0
                    )

            # ---- Phase 1: distances + per-image minima + top-28 means
            if sidx >= 2:
              with (
                tc.tile_pool(name="q2p", bufs=2) as q2p,
                tc.tile_pool(name="quadp", bufs=2, space="PSUM") as quadp,
                tc.tile_pool(name="cpp", bufs=3) as cpp,
                tc.tile_pool(name="foldp", bufs=2) as foldp,
                tc.tile_pool(name="maxbp", bufs=2) as maxbp,
                tc.tile_pool(name="smallp", bufs=2) as smallp,
                tc.tile_pool(name="treep", bufs=2) as treep,
                tc.tile_pool(name="bsubp", bufs=2) as bsubp,
              ):
                def tree_fold(srct, mslice):
                    v = srct.rearrange("p (g two x) -> p g two x", g=8, two=2)
                    f = treep.tile([128, 8, 128], RDT, tag="t128")
                    nc.vector.tensor_tensor(f, v[:, :, 0, :], v[:, :, 1, :], op=OP.max)
                    for w in (64, 32, 16, 8):
                        nxt = treep.tile([128, 8, w], RDT, tag=f"t{w}")
                        vv = f.rearrange("p g (two x) -> p g two x", two=2)
                        nc.vector.tensor_tensor(
                            nxt, vv[:, :, 0, :], vv[:, :, 1, :], op=OP.max
                        )
                        f = nxt
                    nc.vector.tensor_reduce(mslice, f, axis=AX, op=OP.max)

                from contextlib import nullcontext
                loop_cm = (
                    tc.For_i(0, repeat_main, 1) if repeat_main > 1 else nullcontext()
                )
                with loop_cm:
                    for i in range(IPC):
                        q2 = q2p.tile([128, L], MDT, tag="q2")
                        nc.scalar.mul(q2, ZT[:, L * i : L * (i + 1)], 2.0)
                        for h in range(2):
                            lhsT = q2[:, 128 * h : 128 * (h + 1)]
                            maxB = maxbp.tile([128, N], RDT, tag="maxB")
                            for qd in range(NQ):
                                path = quad_paths[qd]
                                need_r1 = path in "DAR"
                                quad = quadp.tile([128, 4, 512], F32, tag="quad")
                                if need_r1:
                                    for j in range(4):
                                        s = 4 * qd + j
                                        nc.tensor.matmul(
                                            quad[:, j, :],
                                            lhsT=ones_inv,
                                            rhs=nsq_rep[
                                                :, 512 * s : 512 * (s + 1)
                                            ],
                                            start=True,
                                            stop=False,
                                        )
                                for j in range(4):
                                    s = 4 * qd + j
                                    nc.tensor.matmul(
                                        quad[:, j, :],
                                        lhsT=lhsT,
                                        rhs=ZT[:, 512 * s : 512 * (s + 1)],
                                        start=not need_r1,
                                        stop=True,
                                    )
                                mslice = maxB[:, 8 * qd : 8 * qd + 8]
                                qflat = quad.rearrange("p f x -> p (f x)")
                                if ablate == "nored":
                                    if qd == 0:
                                        nc.vector.memset(maxB, -100.0)
                                elif path == "D":
                                    nc.vector.tensor_reduce(
                                        mslice,
                                        qflat.rearrange("p (g x) -> p g x", g=8),
                                        axis=AX,
                                        op=OP.max,
                                    )
                                elif path == "A":
                                    cp = cpp.tile([128, 2048], RDT, tag="cp")
                                    nc.scalar.copy(cp, qflat)
                                    nc.vector.tensor_reduce(
                                        mslice,
                                        cp.rearrange("p (g x) -> p g x", g=8),
                                        axis=AX,
                                        op=OP.max,
                                    )
                                else:  # R or C: ACT copy + (C: add -|z|^2) + tree
                                    cp = cpp.tile([128, 2048], RDT, tag="cp")
                                    nc.scalar.copy(cp, qflat)
                                    if path == "C":
                                        b2 = bsubp.tile(
                                            [128, 2048], RDT, tag="bsub"
                                        )
                                        nc.vector.tensor_add(
                                            b2,
                                            cp,
                                            nsq_rep[
                                                :, 2048 * qd : 2048 * (qd + 1)
                                            ],
                                        )
                                        srct = b2
                                    else:
                                        srct = cp
                                    tree_fold(srct, mslice)
                            # finalize (i, h): x = min(maxB - sq_q, 0) = -d2c
                            col = 2 * i + h
                            if ablate == "nofin":
                                nc.vector.tensor_scalar(
                                    score_all[:, col : col + 1],
                                    maxB[:, 0:1],
                                    1.0,
                                    None,
                                    op0=OP.mult,
                                )
                                continue
                            x = smallp.tile([128, N], F32, tag="x")
                            nc.vector.tensor_scalar(
                                x,
                                maxB,
                                sq_q[:, col : col + 1],
                                0.0,
                                op0=OP.subtract,
                                op1=OP.min,
                            )
                            nc.vector.memset(x[:, i : i + 1], NEG)
                            b8 = smallp.tile([128, 32], F32, tag="b8")
                            for r in range(4):
                                nc.vector.max(b8[:, 8 * r : 8 * r + 8], x)
                                if r < 3:
                                    nc.vector.match_replace(
                                        x,
                                        in_to_replace=b8[:, 8 * r : 8 * r + 8],
                                        in_values=x,
                                        imm_value=NEG,
                                    )
                            sv = smallp.tile([128, KTOP], F32, tag="sv")
                            nc.scalar.activation(
                                sv, b8[:, 0:KTOP], ACTF.Sqrt, bias=epsb, scale=-1.0
                            )
                            s28 = smallp.tile([128, 1], F32, tag="s28")
                            nc.vector.reduce_sum(s28, sv, axis=AX)
                            nc.vector.tensor_scalar(
                                score_all[:, col : col + 1],
                                s28,
                                1.0 / KTOP,
                                None,
                                op0=OP.mult,
                            )

            # ---- Phase 2: image scores + AllGather
            if sidx >= 3:
              with tc.tile_pool(name="p2", bufs=1) as p2:
                red = p2.tile([128, 2 * IPC], F32)
                nc.gpsimd.partition_all_reduce(
                    red, score_all, channels=128, reduce_op=bass_isa.ReduceOp.max
                )
                img12 = p2.tile([1, IPC], F32)
                nc.vector.tensor_reduce(
                    img12,
                    red[0:1, :].rearrange("p (i h) -> p i h", h=2),
                    axis=AX,
                    op=OP.max,
                )
                nc.sync.dma_start(cc_in.ap(), img12)
                nc.gpsimd.collective_compute(
                    "AllGather",
                    OP.bypass,
                    replica_groups=[list(range(NCORES))],
                    ins=[cc_in.ap()],
                    outs=[cc_out.ap()],
                )
                nc.sync.dma_start(simg, cc_out.ap())

            # ---- Phase 3: RsCIN / MMO (redundant on every core)
            if sidx >= 4:
              with (
                tc.tile_pool(name="p3", bufs=1) as p3,
                tc.tile_pool(name="p3psum", bufs=2, space="PSUM") as p3p,
              ):
                mn = p3.tile([1, 1], F32)
                mx = p3.tile([1, 1], F32)
                nc.vector.tensor_reduce(mn, simg, axis=AX, op=OP.min)
                nc.vector.tensor_reduce(mx, simg, axis=AX, op=OP.max)
                rngv = p3.tile([1, 1], F32)
                nc.vector.tensor_sub(rngv, mx, mn)
                rcp = p3.tile([1, 1], F32)
                nc.vector.reciprocal(rcp, rngv)
                s_norm = p3.tile([1, N], F32)
                nc.vector.tensor_scalar(
                    s_norm, simg, mn, rcp, op0=OP.subtract, op1=OP.mult
                )
                s_rep = p3.tile([N, N], F32)
                nc.gpsimd.partition_broadcast(s_rep, s_norm, channels=N)

                cls_sb = p3.tile([N, DC], F32)
                nc.sync.dma_start(cls_sb, cls.ap())
                clsT = p3.tile([128, DC // 128, N], F32)
                for d in range(DC // 128):
                    pt = p3p.tile([128, N], F32, tag="pt3")
                    nc.tensor.transpose(
                        pt, cls_sb[:, 128 * d : 128 * (d + 1)], ident[0:N, 0:N]
                    )
                    nc.scalar.copy(clsT[:, d, :], pt)
                Wp = p3p.tile([N, N], F32, tag="Wp")
                for d in range(DC // 128):
                    nc.tensor.matmul(
                        Wp,
                        lhsT=clsT[:, d, :],
                        rhs=clsT[:, d, :],
                        start=(d == 0),
                        stop=(d == DC // 128 - 1),
                    )
                W = p3.tile([N, N], F32)
                nc.scalar.copy(W, Wp)
                m8w = p3.tile([N, 8], F32)
                nc.vector.max(m8w, W)
                acc = p3.tile([N, 1], F32)
                nc.vector.memset(acc, 0.0)
                Wm = p3.tile([N, N], F32)
                Pk = p3.tile([N, N], F32)
                for k in (1, 2, 3):
                    rs = p3.tile([N, 1], F32, tag=f"rs{k}")
                    nc.vector.scalar_tensor_tensor(
                        out=Wm,
                        in0=W,
                        scalar=m8w[:, k - 1 : k],
                        in1=W,
                        op0=OP.is_ge,
                        op1=OP.mult,
                        accum_out=rs,
                    )
                    rck = p3.tile([N, 1], F32, tag=f"rck{k}")
                    nc.vector.reciprocal(rck, rs)
                    Sk = p3.tile([N, 1], F32, tag=f"Sk{k}")
                    nc.vector.tensor_mul(Pk, Wm, s_rep)
                    nc.vector.reduce_sum(Sk, Pk, axis=AX)
                    term = p3.tile([N, 1], F32, tag=f"term{k}")
                    nc.vector.tensor_scalar(term, Sk, rck, None, op0=OP.mult)
                    nc.vector.tensor_add(acc, acc, term)
                out_sb = p3.tile([N, 1], F32)
                nc.vector.tensor_scalar(
                    out_sb, acc, 1.0 / 3.0, None, op0=OP.mult
                )
                nc.sync.dma_start(out.ap(), out_sb)
            if sidx < 4:
                with tc.tile_pool(name="dbg", bufs=1) as dbg:
                    dt_ = dbg.tile([1, N], F32)
                    src_ap = score_all[0:1, 0:24] if sidx >= 2 else ZT[0:1, 0:24]
                    nc.vector.tensor_scalar(
                        dt_[:, 0:24], src_ap.bitcast(F32), 1.0, None, op0=OP.mult
                    )
                    nc.vector.memset(dt_[:, 24:N], 0.0)
                    nc.sync.dma_start(out.ap(), dt_)

    nc.finalize()
    return nc


_CACHE: dict = {}


def _get_nc():
    key = (QUAD_PATHS,)
    if key not in _CACHE:
        _CACHE[key] = build(QUAD_PATHS)
    return _CACHE[key]


def kernel(Z: np.ndarray, cls_tokens: np.ndarray) -> np.ndarray:
    assert Z.shape == (N, L, C) and cls_tokens.shape == (N, DC)
    Z = np.ascontiguousarray(Z, dtype=np.float32)
    cls_tokens = np.ascontiguousarray(cls_tokens, dtype=np.float32)
    nc = _get_nc()
    in_maps = [
        {"Z": np.ascontiguousarray(np.roll(Z, -IPC * c, axis=0)), "cls_tokens": cls_tokens}
        for c in range(NCORES)
    ]
    res = bass_utils.run_bass_kernel_spmd(nc, in_maps, core_ids=list(range(NCORES)))
    return np.asarray(res.results[0]["out"], dtype=np.float32)


if __name__ == "__main__":
    rng = np.random.default_rng(0)
    Zv = rng.standard_normal((N, L, C), dtype=np.float32)
    cv = rng.standard_normal((N, DC), dtype=np.float32)
    print(kernel(Zv, cv)[:8])



# revision 3
# speedup vs baseline: 1.3387x; 1.3387x over previous
"""Trainium2 Bass kernel for nn_BatchMuSc (retrieval_knn) — telescoped rewrite.

Same contract as kernel.py (baseline), but phase 1 uses PSUM-resident
telescoping: for each group of 8 PSUM banks holding B = 2q.z - |z|^2 for 4096
refs, steps advance the query tile by accumulating 2(q_t - q_{t-1}).z via a
single fp32r matmul per stripe.  The -|z|^2 init is paid once per pass
(lhsT = -1 matmul contracting z^2 stripes) instead of once per query tile,
halving tensor-engine work and freeing numerics (no fp16 rank-1).

Drains (PSUM -> per-image max) run on ACT+DVE only (TRN2: at most ONE PSUM
operand per DVE op; GPSIMD implements neither TensorTensor nor PSUM access).
Step-level schedule letters:
  A: both quads ACT-copied to fp16, one batched DVE fold cascade
  M: one quad ACT-copied (cascade), one DVE tensor_reduce direct from PSUM
  D: both quads DVE tensor_reduce direct from PSUM
"""
import os
import sys
import types

import numpy as np

for _p in ("/opt/trn_rl_repo",):
    if _p not in sys.path and os.path.isdir(_p):
        sys.path.insert(0, _p)

try:  # pragma: no cover
    import antenv.axon_hooks  # noqa: F401
except Exception:  # pragma: no cover
    _m = types.ModuleType("antenv.axon_hooks")
    _m.get_axon_ntff_profile_hook = lambda: None
    sys.modules["antenv.axon_hooks"] = _m

import concourse.bacc as bacc
import concourse.bass_isa as bass_isa
import concourse.mybir as mybir
from concourse import bass_utils
from concourse.masks import make_identity
from concourse.tile import TileContext

F32 = mybir.dt.float32
F32R = mybir.dt.float32r
FP16 = mybir.dt.float16
AX = mybir.AxisListType.X
OP = mybir.AluOpType
ACTF = mybir.ActivationFunctionType

N, L, C, DC = 96, 256, 128, 768
NCORES = 8
IPC = N // NCORES            # 12 query images per core
NL = N * L                   # 24576 patches
NT = NL // 128               # 192 transpose tiles
NS = NL // 512               # 48 stripes
NPASS = 8                    # ref passes: 6 stripes (3072 refs, 12 images) each
STEPS = 2 * IPC              # 24 query tiles of 128 (t = 2i + h)
KTOP = 28
EPS = 1e-12
NEG = -3.0e38

SCHED = os.environ.get("BMS2_SCHED", "AAM")
MM_DT = os.environ.get("BMS2_MM_DT", "f32r")   # f32r | fp16


def build(
    repeat_all: int = 1,
    repeat_main: int = 1,
    n_cores: int = NCORES,
    stop: str = "full",
    dummy: bool = False,
    fake_cc: bool = False,
    sched: str = None,
    mm_dt: str = None,
):
    from contextlib import nullcontext

    sched = SCHED if sched is None else sched
    mm_dt = MM_DT if mm_dt is None else mm_dt
    assert set(sched) <= set("AMD")
    f32r = mm_dt == "f32r"
    ZDT = F32R if f32r else FP16

    nc = bacc.Bacc(
        "TRN2",
        target_bir_lowering=False,
        debug=False,
        enable_asserts=False,
        num_devices=n_cores,
    )
    Z = None if dummy else nc.dram_tensor("Z", [N, L, C], F32, kind="ExternalInput")
    cls = nc.dram_tensor("cls_tokens", [N, DC], F32, kind="ExternalInput")
    out = nc.dram_tensor("out", [N], F32, kind="ExternalOutput")
    cc_in = nc.dram_tensor("cc_in", [IPC], F32, kind="Internal")
    cc_out = nc.dram_tensor("cc_out", [N], F32, kind="Internal", addr_space="Shared")

    stages = ["p0", "p1", "full"]
    sidx = stages.index(stop)

    with TileContext(nc) as tc:
      _all_cm = tc.For_i(0, repeat_all, 1) if repeat_all > 1 else nullcontext()
      with _all_cm:
        with tc.tile_pool(name="persist", bufs=1) as pers:
            ident = pers.tile([128, 128], F32)
            make_identity(nc, ident)
            negones = pers.tile([128, 128], FP16)
            nc.vector.memset(negones, -1.0)
            epsb = pers.tile([128, 1], F32)
            nc.vector.memset(epsb, EPS)

            ZT = pers.tile([128, NL], ZDT)       # channels x patches
            zsq = pers.tile([128, NL], FP16)     # z^2 elementwise, ZT layout
            T_all = pers.tile([128, STEPS, 128], ZDT)  # step lhsT tiles
            sq_q = pers.tile([128, STEPS], F32)  # |q|^2 per query tile col
            maxB = pers.tile([128, STEPS, N], FP16)
            score_all = pers.tile([128, STEPS], F32)
            simg = pers.tile([1, N], F32)
            Wt = pers.tile([N, N], F32)
            Wm3 = pers.tile([N, 3, N], F32)
            rck3 = pers.tile([N, 3], F32)

            # ---- Phase 0: load Z, transpose into ZT, z^2, |q|^2, step tiles
            if dummy:
                if f32r:
                    nc.vector.memset(ZT, 0.5)
                    nc.vector.memset(T_all, 0.01)
                else:
                    nc.vector.memset(ZT, 0.5)
                    nc.vector.memset(T_all, 0.01)
                nc.vector.memset(zsq, 0.25)
                nc.vector.memset(sq_q, float(C) * 0.25)
            if not dummy:
                Zf = Z.ap().rearrange("n l c -> (n l) c")
                Zs = Zf.rearrange("(s four p) c -> s p four c", four=4, p=128)

            # ---- Phase 0/1 interleaved: stripe loads feed telescoped passes.
            # SPP stripes per pass; tpsum (2 banks) + tele (6 banks) = 8.
            SPP = NS // NPASS            # stripes per pass
            IPP = 2 * SPP                # images per pass
            IPG = SPP                    # images per group (quad)
            with (
                tc.tile_pool(name="stage", bufs=3) as stage,
                tc.tile_pool(name="tpsum", bufs=2, space="PSUM") as tps,
                tc.tile_pool(name="sqscr", bufs=2) as sqscr,
            ):
              def emit_stripe(s):
                  # load 512 patches, transpose to ZT, z^2, query norms
                  st = stage.tile([128, 4, C], F32, tag="st", name=f"st{s}")
                  nc.sync.dma_start(st, Zs[s])
                  pt = tps.tile([128, 4, 128], F32, tag="pt", name=f"pt{s}")
                  for k in range(4):
                      nc.tensor.transpose(pt[:, k, :], st[:, k, :], ident)
                      if 4 * s + k < STEPS:
                          t = 4 * s + k
                          dm = sqscr.tile([128, C], F32, tag="dm", name=f"dm{t}")
                          nc.scalar.activation(
                              dm, st[:, k, :], ACTF.Square,
                              accum_out=sq_q[:, t : t + 1],
                          )
                  sl = slice(512 * s, 512 * (s + 1))
                  ptf = pt.rearrange("p f x -> p (f x)")
                  if s % 2 == 0:
                      nc.scalar.copy(ZT[:, sl], ptf)
                      nc.vector.tensor_mul(zsq[:, sl], ZT[:, sl], ZT[:, sl])
                  else:
                      nc.vector.tensor_copy(ZT[:, sl], ptf)
                      nc.scalar.activation(zsq[:, sl], ZT[:, sl], ACTF.Square)

              if not dummy:
                # prologue: query stripes + step tiles
                nprew = NS if sidx < 1 else SPP
                for s in range(nprew):
                    emit_stripe(s)
                with tc.tile_pool(name="qscr", bufs=1) as qscr:
                    q2 = qscr.tile([128, STEPS, 128], ZDT)
                    nc.vector.tensor_scalar(
                        q2.rearrange("p t x -> p (t x)"),
                        ZT[:, 0 : STEPS * 128],
                        2.0,
                        None,
                        op0=OP.mult,
                    )
                    nc.vector.tensor_copy(T_all[:, 0, :], q2[:, 0, :])
                    nc.vector.tensor_sub(
                        T_all.rearrange("p t x -> p (t x)")[:, 128:],
                        q2.rearrange("p t x -> p (t x)")[:, 128:],
                        q2.rearrange("p t x -> p (t x)")[:, : (STEPS - 1) * 128],
                    )

              if sidx >= 2:
                # MMO cls-prep: W, top-k masks, row-sum reciprocals
                with (
                  tc.tile_pool(name="clsprep", bufs=1) as cpr,
                  tc.tile_pool(name="clspsum", bufs=2, space="PSUM") as cps,
                ):
                  cls_sb = cpr.tile([N, DC], F32)
                  nc.sync.dma_start(cls_sb, cls.ap())
                  clsT = cpr.tile([128, DC // 128, N], F32)
                  for d in range(DC // 128):
                      ptc = cps.tile([128, N], F32, tag="ptc", name=f"ptc{d}")
                      nc.tensor.transpose(
                          ptc,
                          cls_sb[:, 128 * d : 128 * (d + 1)],
                          ident[0:N, 0:N],
                      )
                      nc.scalar.copy(clsT[:, d, :], ptc)
                  Wp = cps.tile([N, N], F32, tag="wp", name="wp")
                  for d in range(DC // 128):
                      nc.tensor.matmul(
                          Wp,
                          lhsT=clsT[:, d, :],
                          rhs=clsT[:, d, :],
                          start=(d == 0),
                          stop=(d == DC // 128 - 1),
                      )
                  nc.scalar.copy(Wt, Wp)
                  m8w = cpr.tile([N, 8], F32)
                  nc.vector.max(m8w, Wt)
                  for k in (1, 2, 3):
                      rs = cpr.tile([N, 1], F32, tag=f"rs{k}", name=f"rs{k}")
                      nc.vector.scalar_tensor_tensor(
                          out=Wm3[:, k - 1, :],
                          in0=Wt,
                          scalar=m8w[:, k - 1 : k],
                          in1=Wt,
                          op0=OP.is_ge,
                          op1=OP.mult,
                          accum_out=rs,
                      )
                      nc.vector.reciprocal(rck3[:, k - 1 : k], rs)

              # ---- telescoped distance accumulation + drains
              if sidx >= 1:
                ZT_mm = ZT
                T_mm = T_all
                with (
                  tc.tile_pool(name="tele", bufs=1, space="PSUM") as tele,
                  tc.tile_pool(name="cpp", bufs=2) as cpp,
                  tc.tile_pool(name="treep", bufs=1) as treep,
                  tc.tile_pool(name="finp", bufs=2) as finp,
                ):
                  loop_cm = (
                      tc.For_i(0, repeat_main, 1) if repeat_main > 1
                      else nullcontext()
                  )
                  with loop_cm:
                    qi = 0
                    for p in range(NPASS):
                      Qg = [
                          tele.tile([128, 3, 512], F32, tag=f"q{g}",
                                    name=f"q{g}_{p}")
                          for g in range(2)
                      ]
                      for g in range(2):
                          for j in range(3):
                              s = SPP * p + 3 * g + j
                              nc.tensor.matmul(
                                  Qg[g][:, j, :],
                                  lhsT=negones,
                                  rhs=zsq[:, 512 * s : 512 * (s + 1)],
                                  start=True,
                                  stop=False,
                                  skip_group_check=True,
                              )

                      def cascade(cp, qlo, qhi, tlist):
                          # fp16 fold cascade over cp[:, qlo:qhi] -> maxB rows
                          nq = qhi - qlo
                          cpv = cp[:, qlo:qhi, :]
                          cg2 = cpv.rearrange(
                              "p q (g two x) -> p q g two x", g=IPG, two=2
                          )
                          f1 = treep.tile([128, 4, IPG, 128], FP16, tag="f1")
                          nc.vector.tensor_tensor(
                              f1[:, 0:nq], cg2[:, :, :, 0, :], cg2[:, :, :, 1, :],
                              op=OP.max,
                          )
                          f2 = treep.tile([128, 4, IPG, 64], FP16, tag="f2")
                          v1 = f1[:, 0:nq].rearrange(
                              "p q g (two x) -> p q g two x", two=2
                          )
                          nc.vector.tensor_tensor(
                              f2[:, 0:nq], v1[:, :, :, 0, :], v1[:, :, :, 1, :],
                              op=OP.max,
                          )
                          f3 = treep.tile([128, 4, IPG, 32], FP16, tag="f3")
                          v2 = f2[:, 0:nq].rearrange(
                              "p q g (two x) -> p q g two x", two=2
                          )
                          nc.vector.tensor_tensor(
                              f3[:, 0:nq], v2[:, :, :, 0, :], v2[:, :, :, 1, :],
                              op=OP.max,
                          )
                          c0 = IPP * p
                          if len(tlist) == 2:
                              assert tlist[1] == tlist[0] + 1
                              dst = maxB[
                                  :, tlist[0] : tlist[0] + 2, c0 : c0 + 2 * IPG
                              ].rearrange("p t (q g) -> p t q g", g=IPG)
                              src = f3[:, 0:nq].rearrange(
                                  "p (t q) g x -> p t q g x", t=2
                              )
                          else:
                              dst = maxB[
                                  :, tlist[0], c0 : c0 + nq * IPG
                              ].rearrange("p (q g) -> p q g", g=IPG)
                              src = f3[:, 0:nq]
                          nc.vector.tensor_reduce(dst, src, axis=AX, op=OP.max)

                      pend = None  # (cp tile, first A-step t)
                      for t in range(STEPS):
                        # preload next pass's stripes between steps
                        if not dummy and p < NPASS - 1 and t % 4 == 1 and t // 4 < SPP:
                            emit_stripe(SPP * (p + 1) + t // 4)
                        ch = sched[qi % len(sched)]
                        qi += 1
                        ncopy = {"A": 2, "M": 1, "D": 0}[ch]
                        if ncopy == 2 and pend is None:
                            cp = cpp.tile([128, 4, 1536], FP16, tag="cp")
                            qbase = 0
                        elif ncopy == 2:
                            cp = pend[0]
                            qbase = 2
                        elif ncopy == 1:
                            cp = cpp.tile([128, 4, 1536], FP16, tag="cp")
                            qbase = 0
                        for g in range(2):
                            Q = Qg[g]
                            for j in range(3):
                                s = SPP * p + 3 * g + j
                                nc.tensor.matmul(
                                    Q[:, j, :],
                                    lhsT=T_mm[:, t, :],
                                    rhs=ZT_mm[:, 512 * s : 512 * (s + 1)],
                                    start=False,
                                    stop=(t == STEPS - 1),
                                    skip_group_check=True,
                                )
                            qflat = Q.rearrange("p f x -> p (f x)")
                            if g < ncopy:
                                nc.scalar.copy(cp[:, qbase + g, :], qflat)
                            else:
                                # DVE grouped reduce direct from PSUM
                                c0 = IPP * p + IPG * g
                                nc.vector.tensor_reduce(
                                    maxB[:, t, c0 : c0 + IPG],
                                    qflat.rearrange("p (g x) -> p g x", g=IPG),
                                    axis=AX,
                                    op=OP.max,
                                )
                        completed = []
                        if ncopy == 2:
                            if pend is None:
                                pend = (cp, t)
                            else:
                                if t == pend[1] + 1:
                                    cascade(cp, 0, 4, [pend[1], t])
                                else:
                                    cascade(cp, 0, 2, [pend[1]])
                                    cascade(cp, 2, 4, [t])
                                completed = [pend[1], t]
                                pend = None
                        elif ncopy == 1:
                            cascade(cp, 0, 1, [t])
                            completed = [t]
                        else:
                            completed = [t]
                        if pend is not None and t == STEPS - 1:
                            cascade(pend[0], 0, 2, [pend[1]])
                            completed.append(pend[1])
                            pend = None
                        # finalize steps whose maxB row is now complete
                        if p == NPASS - 1:
                          for t in completed:
                            x = finp.tile([128, N], F32, tag="x")
                            nc.vector.tensor_scalar(
                                x,
                                maxB[:, t, :],
                                sq_q[:, t : t + 1],
                                None,
                                op0=OP.subtract,
                            )
                            nc.vector.memset(x[:, t // 2 : t // 2 + 1], NEG)
                            b8 = finp.tile([128, 32], F32, tag="b8")
                            for r in range(4):
                                nc.vector.max(b8[:, 8 * r : 8 * r + 8], x)
                                if r < 3:
                                    nc.vector.match_replace(
                                        x,
                                        in_to_replace=b8[:, 8 * r : 8 * r + 8],
                                        in_values=x,
                                        imm_value=NEG,
                                    )
                            sv = finp.tile([128, KTOP], F32, tag="sv")
                            nc.scalar.activation(
                                sv,
                                b8[:, 0:KTOP],
                                ACTF.Sqrt,
                                bias=epsb,
                                scale=-1.0,
                                accum_out=score_all[:, t : t + 1],
                            )
              # score_all currently holds sum of 28 sqrt distances; scale later

            # ---- Phase 2: image scores + AllGather
            if sidx >= 2:
              with tc.tile_pool(name="p2", bufs=1) as p2:
                red = p2.tile([128, STEPS], F32)
                nc.gpsimd.partition_all_reduce(
                    red, score_all, channels=128, reduce_op=bass_isa.ReduceOp.max
                )
                img12 = p2.tile([1, IPC], F32)
                nc.vector.tensor_reduce(
                    img12,
                    red[0:1, :].rearrange("p (i h) -> p i h", h=2),
                    axis=AX,
                    op=OP.max,
                )
                img12s = p2.tile([1, IPC], F32)
                nc.vector.tensor_scalar(
                    img12s, img12, 1.0 / KTOP, None, op0=OP.mult
                )
                nc.sync.dma_start(cc_in.ap(), img12s)
                if fake_cc:
                    nc.sync.dma_start(cc_out.ap()[0:IPC], cc_in.ap())
                else:
                    nc.gpsimd.collective_compute(
                        "AllGather",
                        OP.bypass,
                        replica_groups=[list(range(n_cores))],
                        ins=[cc_in.ap()],
                        outs=[cc_out.ap()],
                    )
                nc.sync.dma_start(simg, cc_out.ap())

              # ---- Phase 3: RsCIN tail (W/masks/row-sums precomputed)
              with tc.tile_pool(name="p3", bufs=1) as p3:
                mn = p3.tile([1, 1], F32)
                mx = p3.tile([1, 1], F32)
                nc.vector.tensor_reduce(mn, simg, axis=AX, op=OP.min)
                nc.vector.tensor_reduce(mx, simg, axis=AX, op=OP.max)
                rngv = p3.tile([1, 1], F32)
                nc.vector.tensor_sub(rngv, mx, mn)
                rcp = p3.tile([1, 1], F32)
                nc.vector.reciprocal(rcp, rngv)
                s_norm = p3.tile([1, N], F32)
                nc.vector.tensor_scalar(
                    s_norm, simg, mn, rcp, op0=OP.subtract, op1=OP.mult
                )
                s_rep = p3.tile([N, N], F32)
                nc.gpsimd.partition_broadcast(s_rep, s_norm, channels=N)

                acc = p3.tile([N, 1], F32)
                nc.vector.memset(acc, 0.0)
                Pk = p3.tile([N, N], F32)
                for k in (1, 2, 3):
                    Sk = p3.tile([N, 1], F32, tag=f"Sk{k}", name=f"Sk{k}")
                    nc.vector.tensor_mul(Pk, Wm3[:, k - 1, :], s_rep)
                    nc.vector.reduce_sum(Sk, Pk, axis=AX)
                    term = p3.tile([N, 1], F32, tag=f"term{k}", name=f"term{k}")
                    nc.vector.tensor_scalar(
                        term, Sk, rck3[:, k - 1 : k], None, op0=OP.mult
                    )
                    nc.vector.tensor_add(acc, acc, term)
                out_sb = p3.tile([N, 1], F32)
                nc.vector.tensor_scalar(
                    out_sb, acc, 1.0 / 3.0, None, op0=OP.mult
                )
                nc.sync.dma_start(out.ap(), out_sb)
            else:
                with tc.tile_pool(name="dbg", bufs=1) as dbg:
                    dt_ = dbg.tile([1, N], F32)
                    src_ap = score_all[0:1, 0:24] if sidx >= 1 else sq_q[0:1, 0:24]
                    nc.vector.tensor_scalar(
                        dt_[:, 0:24], src_ap, 1.0, None, op0=OP.mult
                    )
                    nc.vector.memset(dt_[:, 24:N], 0.0)
                    nc.sync.dma_start(out.ap(), dt_)

    nc.finalize()
    return nc


_CACHE: dict = {}


def _get_nc():
    key = (SCHED, MM_DT)
    if key not in _CACHE:
        _CACHE[key] = build()
    return _CACHE[key]


def kernel(Z: np.ndarray, cls_tokens: np.ndarray) -> np.ndarray:
    assert Z.shape == (N, L, C) and cls_tokens.shape == (N, DC)
    Z = np.ascontiguousarray(Z, dtype=np.float32)
    cls_tokens = np.ascontiguousarray(cls_tokens, dtype=np.float32)
    nc = _get_nc()
    in_maps = [
        {"Z": np.ascontiguousarray(np.roll(Z, -IPC * c, axis=0)),
         "cls_tokens": cls_tokens}
        for c in range(NCORES)
    ]
    res = bass_utils.run_bass_kernel_spmd(nc, in_maps, core_ids=list(range(NCORES)))
    return np.asarray(res.results[0]["out"], dtype=np.float32)


if __name__ == "__main__":
    rng = np.random.default_rng(0)
    Zv = rng.standard_normal((N, L, C), dtype=np.float32)
    cv = rng.standard_normal((N, DC), dtype=np.float32)
    print(kernel(Zv, cv)[:8])
